# revision 1
# baseline (speedup 1.0000x reference)
"""Multi-head self-attention (B=4, S=2048, E=1024, H=16, causal) on 8 NeuronCores.

Best validated configuration (367.8-377.1us over four HW runs; staged
baseline was 464-549us): attn@V matmuls run one round behind scores/exp so
the in-order PE queue never stalls on ScalarE (stalls re-throttle the HAM
clock to 1.2 GHz); ScalarE does exp only (Q/K bias adds on VectorE, V bias
folded into V pre-attention — exact because softmax rows sum to 1);
softmax normalization broadcasts the denominator with a rank-1 PE matmul,
then inverts the broadcast with the fast custom-DVE reciprocal at base
partition 0 (reciprocal_approx_fast returns garbage on HW at any other base
partition); one projection/out-proj filler group interleaved per attention
round keeps the PE dense; 20 warm-up matmuls cover the input-DMA window.
"""

import numpy as np
import ml_dtypes

B, S, E, H, D = 4, 2048, 1024, 16, 64
HPC = 8          # heads per core
DC = HPC * D     # 512 sharded feature cols per core
EC = E // 128    # 8 e-chunks
TT = S // 128    # 16 token tiles
QCH = S // 512   # 4 query chunks
NB = S // 128    # 16 key blocks

BF16 = ml_dtypes.bfloat16

_CACHE = {}


def _build():
    import concourse.tile as tile
    from concourse import bacc, mybir

    F32 = mybir.dt.float32
    BF = mybir.dt.bfloat16
    AF = mybir.ActivationFunctionType
    ALU = mybir.AluOpType

    nc = bacc.Bacc("TRN2", target_bir_lowering=False, debug=False, num_devices=8)

    xT_d = nc.dram_tensor("xT", [EC, 128, S], BF, kind="ExternalInput")
    wq_d = nc.dram_tensor("wq", [EC, 128, DC], BF, kind="ExternalInput")
    wk_d = nc.dram_tensor("wk", [EC, 128, DC], BF, kind="ExternalInput")
    wv_d = nc.dram_tensor("wv", [EC, 128, DC], BF, kind="ExternalInput")
    wo_d = nc.dram_tensor("wo", [DC // 128, 128, E], BF, kind="ExternalInput")
    bq_d = nc.dram_tensor("bq", [128, 4], F32, kind="ExternalInput")
    bk_d = nc.dram_tensor("bk", [128, 4], F32, kind="ExternalInput")
    bvb_d = nc.dram_tensor("bvb", [128, DC], F32, kind="ExternalInput")
    mask_d = nc.dram_tensor("mask", [128, 128], BF, kind="ExternalInput")
    out_d = nc.dram_tensor("out", [TT, 128, E], F32, kind="ExternalOutput")

    with tile.TileContext(nc) as tc:
        with tc.tile_pool(name="const", bufs=1) as cp, \
             tc.tile_pool(name="expp", bufs=4) as expp, \
             tc.tile_pool(name="work", bufs=2) as wp, \
             tc.tile_pool(name="ps_s", bufs=2, space="PSUM") as ps_s, \
             tc.tile_pool(name="ps_av", bufs=2, space="PSUM") as ps_av, \
             tc.tile_pool(name="ps_w", bufs=2, space="PSUM") as ps_w:

            # ---- PE warm-up during the input-DMA window ----
            wu = cp.tile([64, 512], BF, tag="wu", name="wu")
            nc.vector.memset(wu[:], 0.125)
            for _ in range(20):
                pw = ps_w.tile([128, 512], F32, tag="psw", name="psw")
                nc.tensor.matmul(pw[0:64, :], wu[:, 0:64], wu[:],
                                 start=True, stop=True)

            # ---- persistent SBUF tensors + input DMAs ----
            xT = [cp.tile([128, S], BF, tag=f"xT{k}", name=f"xT{k}") for k in range(EC)]
            wq = [cp.tile([128, DC], BF, tag=f"wq{k}", name=f"wq{k}") for k in range(EC)]
            wk = [cp.tile([128, DC], BF, tag=f"wk{k}", name=f"wk{k}") for k in range(EC)]
            wv = [cp.tile([128, DC], BF, tag=f"wv{k}", name=f"wv{k}") for k in range(EC)]
            wo = [cp.tile([128, E], BF, tag=f"wo{k}", name=f"wo{k}") for k in range(DC // 128)]
            for k in range(EC):
                nc.sync.dma_start(xT[k][:], xT_d.ap()[k])
                nc.gpsimd.dma_start(wq[k][:], wq_d.ap()[k])
                nc.gpsimd.dma_start(wk[k][:], wk_d.ap()[k])
                nc.gpsimd.dma_start(wv[k][:], wv_d.ap()[k])
            for k in range(DC // 128):
                nc.sync.dma_start(wo[k][:], wo_d.ap()[k])
            bq = cp.tile([128, 4], F32, tag="bq", name="bq")
            bk = cp.tile([128, 4], F32, tag="bk", name="bk")
            bvb = cp.tile([128, DC], F32, tag="bvb", name="bvb")
            mask = cp.tile([128, 128], BF, tag="mask", name="mask")
            nc.sync.dma_start(bq[:], bq_d.ap())
            nc.sync.dma_start(bk[:], bk_d.ap())
            nc.sync.dma_start(bvb[:], bvb_d.ap())
            nc.sync.dma_start(mask[:], mask_d.ap())
            ones = cp.tile([65, 64], BF, tag="ones", name="ones")
            nc.any.memset(ones[:], 1.0)

            QT = [cp.tile([128, S], BF, tag=f"QT{t}", name=f"QT{t}") for t in range(4)]
            KT = [cp.tile([128, S], BF, tag=f"KT{t}", name=f"KT{t}") for t in range(4)]
            V = [cp.tile([128, HPC, 66], BF, tag=f"V{s}", name=f"V{s}") for s in range(TT)]
            AOT = [cp.tile([128, S], BF, tag=f"AOT{t}", name=f"AOT{t}") for t in range(4)]

            filler = []

            def proj_group(w_sb, b_sb, dst, t, qc):
                def emit():
                    ps = ps_w.tile([128, 512], F32, tag="psw", name="psw")
                    for k in range(EC):
                        nc.tensor.matmul(
                            ps[:],
                            w_sb[k][:, t * 128:(t + 1) * 128],
                            xT[k][:, qc * 512:(qc + 1) * 512],
                            start=(k == 0), stop=(k == EC - 1))
                    nc.vector.tensor_scalar(
                        dst[t][:, qc * 512:(qc + 1) * 512], ps[:],
                        b_sb[:, t:t + 1], None, ALU.add)
                return emit

            def v_group(s):
                def emit():
                    ps = ps_w.tile([128, 512], F32, tag="psw", name="psw")
                    for k in range(EC):
                        nc.tensor.matmul(
                            ps[:],
                            xT[k][:, s * 128:(s + 1) * 128],
                            wv[k][:],
                            start=(k == 0), stop=(k == EC - 1))
                    nc.vector.tensor_tensor(
                        V[s][:, :, 0:64],
                        ps[:].rearrange("p (h d) -> p h d", d=64),
                        bvb[:].rearrange("p (h d) -> p h d", d=64),
                        ALU.add)
                    nc.any.memset(V[s][:, :, 64:65], 1.0)
                return emit

            def d_group(s):
                def emit():
                    osb = wp.tile([128, E], F32, tag="osb", name="osb")
                    for n in range(2):
                        ps = ps_w.tile([128, 512], F32, tag="psw", name="psw")
                        for k in range(DC // 128):
                            nc.tensor.matmul(
                                ps[:],
                                AOT[k][:, s * 128:(s + 1) * 128],
                                wo[k][:, n * 512:(n + 1) * 512],
                                start=(k == 0), stop=(k == DC // 128 - 1))
                        nc.vector.tensor_copy(out=osb[:, n * 512:(n + 1) * 512],
                                              in_=ps[:])
                    nc.sync.dma_start(out_d.ap()[s], osb[:])
                return emit

            for t in range(4):
                for qc in range(QCH):
                    if t == 0:
                        proj_group(wq, bq, QT, t, qc)()
                        proj_group(wk, bk, KT, t, qc)()
                    else:
                        filler.append(("qkt", t, proj_group(wq, bq, QT, t, qc)))
                        filler.append(("qkt", t, proj_group(wk, bk, KT, t, qc)))
            for s in range(TT):
                if s < 4:
                    v_group(s)()
                else:
                    filler.append(("v", s, v_group(s)))

            # Filler rationing: attention rounds are ScalarE-paced with a
            # ~1.2us/round PE deficit, but a filler group is 1.71us — uniform
            # one-per-round over-fills the early (qc=3) block and starves the
            # late phases, which then idle and re-throttle the HAM clock.
            # Emit by accumulated debt instead, so supply matches need.
            FILL_NS = 1750.0
            debt = [0.0]
            pending_norm = [None]

            def flush_norm():
                if pending_norm[0] is not None:
                    pending_norm[0]()
                    pending_norm[0] = None

            def act_ns(nfree):
                return (nfree + 352) / 1.2

            def emit_filler_until(pred_drop):
                keep = []
                for item in filler:
                    if pred_drop(item):
                        item[2]()
                        debt[0] -= FILL_NS
                    else:
                        keep.append(item)
                filler[:] = keep

            for qc in (3, 0, 1, 2):
                nkb = 4 * qc + 4
                for hp in range(4):
                    emit_filler_until(lambda it: it[0] == "qkt" and it[1] <= hp)
                    hA, hB = 2 * hp, 2 * hp + 1
                    pav = {}
                    pav[hA] = ps_av.tile([128, 512], F32, tag="pav", name="pav")
                    pav[hB] = ps_av.tile([128, 512], F32, tag="pav", name="pav")

                    def emit_av(ex, kbs_offs):
                        for h in (hA, hB):
                            for i, kb, off in kbs_offs:
                                nc.tensor.matmul(
                                    pav[h][0:65, off:512],
                                    V[kb][:, h, 0:65],
                                    ex[h][:, i, off:512],
                                    start=(kb == 0), stop=(kb == nkb - 1))

                    pend = None
                    for s0 in range(0, nkb, 2):
                        kbs = list(range(s0, min(s0 + 2, nkb)))
                        emit_filler_until(
                            lambda it: it[0] == "v" and it[1] <= kbs[-1])
                        pss = {h: ps_s.tile([128, 2, 512], F32, tag="pss",
                                            name="pss")
                               for h in (hA, hB)}
                        ex = {h: expp.tile([128, 2, 512], BF,
                                           tag=f"ex{h % 2}", name="ex")
                              for h in (hA, hB)}
                        # AV/mask offsets are per-kb; the scores matmuls write
                        # the whole round width [off0:512] so ONE exp per
                        # (round, head) reads only data written this round —
                        # halves the diagonal-round ACTIVATE count (the act
                        # fixed cost is 352 cycles). The widened slice of the
                        # second kb holds above-diagonal values nothing reads:
                        # AV reads [off:512] per kb, mask covers the boundary.
                        offs = {}
                        off0 = 128 * (kbs[0] - 4 * qc) if kbs[0] > 4 * qc else 0
                        for i, kb in enumerate(kbs):
                            dj = kb - 4 * qc
                            offs[kb] = 128 * dj if dj > 0 else 0
                            for h, r in ((hA, 0), (hB, 64)):
                                nc.tensor.matmul(
                                    pss[h][:, i, off0:512],
                                    KT[hp][r:r + 64, kb * 128:(kb + 1) * 128],
                                    QT[hp][r:r + 64,
                                           qc * 512 + off0:(qc + 1) * 512],
                                    start=True, stop=True)
                        for h in (hA, hB):
                            nc.scalar.activation(
                                ex[h][:, 0:len(kbs), off0:512],
                                pss[h][:, 0:len(kbs), off0:512],
                                AF.Exp, scale=0.125)
                            for i, kb in enumerate(kbs):
                                if kb - 4 * qc >= 0:
                                    off = offs[kb]
                                    nc.vector.tensor_tensor(
                                        ex[h][:, i, off:off + 128],
                                        ex[h][:, i, off:off + 128],
                                        mask[:], ALU.mult)
                        sc_free = len(kbs) * (512 - off0)
                        debt[0] += 2 * act_ns(sc_free) \
                            - (0.55 * sc_free + 2 * sc_free) / 2.4
                        while debt[0] >= FILL_NS and filler:
                            filler.pop(0)[2]()
                            debt[0] -= FILL_NS
                        flush_norm()
                        if pend is not None:
                            emit_av(*pend)
                        pend = (ex, [(i, kb, offs[kb])
                                     for i, kb in enumerate(kbs)])
                    emit_av(*pend)

                    # defer this phase's normalization into the next phase's
                    # first round: the den-cast -> broadcast chain otherwise
                    # stalls the PE ~1.1us at every phase boundary (and each
                    # stall re-throttles the HAM clock for 4-6us)
                    def make_norm(qc, hp, hA, hB, pav):
                        def emit():
                            for h, r in ((hA, 0), (hB, 64)):
                                den = wp.tile([1, 512], BF, tag="den",
                                              name="den")
                                nc.vector.tensor_copy(out=den[:],
                                                      in_=pav[h][64:65, :])
                                psb = ps_w.tile([128, 512], F32, tag="psw",
                                                name="psw")
                                nc.tensor.matmul(psb[0:64, :], ones[0:1, :],
                                                 den[:], start=True, stop=True)
                                rcpb = wp.tile([64, 512], F32, tag="rcpb",
                                               name="rcpb")
                                nc.vector.reciprocal_approx_fast(
                                    out=rcpb[:], in_=psb[0:64, :])
                                dst = AOT[hp][r:r + 64,
                                              qc * 512:(qc + 1) * 512]
                                nc.vector.tensor_tensor(dst, pav[h][0:64, :],
                                                        rcpb[:], ALU.mult)
                        return emit
                    pending_norm[0] = make_norm(qc, hp, hA, hB, pav)
                # d_group(s) needs the AOT of all four head-pairs, so the
                # last phase's pending norm must flush before it runs
                def d_filler(s):
                    def emit():
                        flush_norm()
                        d_group(s)()
                    return emit
                for s in range(qc * 4, qc * 4 + 4):
                    filler.append(("d", s, d_filler(s)))
            flush_norm()
            emit_filler_until(lambda it: True)

    nc.compile()
    return nc


def _get_nc():
    if "nc" not in _CACHE:
        _CACHE["nc"] = _build()
    return _CACHE["nc"]


def _shard_inputs(x, Wq, bq, Wk, bk, Wv, bv, Wo):
    """Build the 8 per-core input maps (host-side shard/cast/transpose)."""
    x = np.asarray(x, np.float32)
    mask = np.triu(np.ones((128, 128), np.float32)).astype(BF16)  # [k, q] q>=k
    in_maps = []
    for c in range(8):
        b, hg = divmod(c, 2)
        dc = slice(hg * DC, (hg + 1) * DC)
        xT = np.ascontiguousarray(x[b].T).astype(BF16).reshape(EC, 128, S)
        wq_c = np.ascontiguousarray(Wq[:, dc]).astype(BF16).reshape(EC, 128, DC)
        wk_c = np.ascontiguousarray(Wk[:, dc]).astype(BF16).reshape(EC, 128, DC)
        wv_c = np.ascontiguousarray(Wv[:, dc]).astype(BF16).reshape(EC, 128, DC)
        wo_c = np.ascontiguousarray(Wo[dc, :]).astype(BF16).reshape(DC // 128, 128, E)
        bq_c = np.ascontiguousarray(np.asarray(bq[dc], np.float32).reshape(4, 128).T)
        bk_c = np.ascontiguousarray(np.asarray(bk[dc], np.float32).reshape(4, 128).T)
        bvb_c = np.ascontiguousarray(
            np.tile(np.asarray(bv[dc], np.float32).reshape(1, DC), (128, 1)))
        in_maps.append({
            "xT": xT, "wq": wq_c, "wk": wk_c, "wv": wv_c, "wo": wo_c,
            "bq": bq_c, "bk": bk_c, "bvb": bvb_c, "mask": mask,
        })
    return in_maps


def kernel(x, Wq, bq, Wk, bk, Wv, bv, Wo, bo):
    from concourse.bass_utils import run_bass_kernel_spmd

    nc = _get_nc()
    in_maps = _shard_inputs(x, Wq, bq, Wk, bk, Wv, bv, Wo)
    res = run_bass_kernel_spmd(nc, in_maps, core_ids=list(range(8)))
    bo = np.asarray(bo, np.float32)
    out = np.empty((B, S, E), np.float32)
    for b in range(B):
        p0 = res.results[2 * b]["out"].reshape(S, E)
        p1 = res.results[2 * b + 1]["out"].reshape(S, E)
        out[b] = p0 + p1 + bo
    return out



# revision 3
# speedup vs baseline: 1.0084x; 1.0084x over previous
"""Multi-head self-attention (B=4, S=2048, E=1024, H=16, causal) on 8 NeuronCores.

v2 schedule: phases run in a "snake" order over (head-pair, query-chunk)
so that the Q/K projection, V projection and out-projection filler groups
have deadlines spread across the whole run.  The previous (qc-major)
order forced every projection filler into the first phase, leaving the
ScalarE-paced middle phases with ~50% PE duty -> HAM re-throttled the PE
clock to 1.2 GHz for ~110us.  Other carried-over tricks: attn@V matmuls
run one round behind scores/exp; ScalarE does exp only (Q/K bias adds on
VectorE, V bias folded into V pre-attention); softmax denominator
broadcast via rank-1 PE matmul + custom-DVE reciprocal at base partition
0; per-round filler metering by accumulated ScalarE-vs-PE debt.
"""

import numpy as np
import ml_dtypes

B, S, E, H, D = 4, 2048, 1024, 16, 64
HPC = 8          # heads per core
DC = HPC * D     # 512 sharded feature cols per core
EC = E // 128    # 8 e-chunks
TT = S // 128    # 16 token tiles
QCH = S // 512   # 4 query chunks
NB = S // 128    # 16 key blocks

BF16 = ml_dtypes.bfloat16

_CACHE = {}

# snake order over (hp, qc): spreads qkt[hp] deadlines (first (hp,*) phase),
# v-band deadlines (first (*,qc) phase) and d(qc) readiness (last (*,qc))
PHASES = [(0, 0), (0, 1), (1, 0), (1, 1), (0, 2), (1, 2), (2, 0), (2, 1),
          (2, 2), (0, 3), (1, 3), (2, 3), (3, 0), (3, 1), (3, 2), (3, 3)]


def _build():
    import concourse.tile as tile
    from concourse import bacc, mybir

    F32 = mybir.dt.float32
    BF = mybir.dt.bfloat16
    AF = mybir.ActivationFunctionType
    ALU = mybir.AluOpType

    nc = bacc.Bacc("TRN2", target_bir_lowering=False, debug=False, num_devices=8)

    xT_d = nc.dram_tensor("xT", [EC, 128, S], BF, kind="ExternalInput")
    wq_d = nc.dram_tensor("wq", [EC, 128, DC], BF, kind="ExternalInput")
    wk_d = nc.dram_tensor("wk", [EC, 128, DC], BF, kind="ExternalInput")
    wv_d = nc.dram_tensor("wv", [EC, 128, DC], BF, kind="ExternalInput")
    wo_d = nc.dram_tensor("wo", [DC // 128, 128, E], BF, kind="ExternalInput")
    bq_d = nc.dram_tensor("bq", [128, 4], F32, kind="ExternalInput")
    bk_d = nc.dram_tensor("bk", [128, 4], F32, kind="ExternalInput")
    bvb_d = nc.dram_tensor("bvb", [128, DC], F32, kind="ExternalInput")
    mask_d = nc.dram_tensor("mask", [128, 128], BF, kind="ExternalInput")
    out_d = nc.dram_tensor("out", [TT, 128, E], BF, kind="ExternalOutput")

    with tile.TileContext(nc) as tc:
        with tc.tile_pool(name="const", bufs=1) as cp, \
             tc.tile_pool(name="expp", bufs=4) as expp, \
             tc.tile_pool(name="work", bufs=2) as wp, \
             tc.tile_pool(name="ps_s", bufs=2, space="PSUM") as ps_s, \
             tc.tile_pool(name="ps_av", bufs=2, space="PSUM") as ps_av, \
             tc.tile_pool(name="ps_w", bufs=2, space="PSUM") as ps_w:

            # ---- persistent SBUF tensors ----
            xT = [cp.tile([128, S], BF, tag=f"xT{k}", name=f"xT{k}") for k in range(EC)]
            wq = [cp.tile([128, DC], BF, tag=f"wq{k}", name=f"wq{k}") for k in range(EC)]
            wk = [cp.tile([128, DC], BF, tag=f"wk{k}", name=f"wk{k}") for k in range(EC)]
            wv = [cp.tile([128, DC], BF, tag=f"wv{k}", name=f"wv{k}") for k in range(EC)]
            wo = [cp.tile([128, E], BF, tag=f"wo{k}", name=f"wo{k}") for k in range(DC // 128)]
            bq = cp.tile([128, 4], F32, tag="bq", name="bq")
            bk = cp.tile([128, 4], F32, tag="bk", name="bk")
            bvb = cp.tile([128, DC], F32, tag="bvb", name="bvb")
            mask = cp.tile([128, 128], BF, tag="mask", name="mask")

            # ---- input DMAs on the 3 DMA-capable queues (sync/gpsimd/
            # scalar), each FIFO-ordered by first consumption: lead-in
            # projections need (bq, wq[k], wk[k], xT[k]) chunk-by-chunk ----
            nc.sync.dma_start(bq[:], bq_d.ap())
            nc.sync.dma_start(bk[:], bk_d.ap())
            for k in range(EC):
                nc.sync.dma_start(xT[k][:], xT_d.ap()[k])
            for k in range(EC):
                nc.gpsimd.dma_start(wq[k][:], wq_d.ap()[k])
            for k in range(EC):
                nc.scalar.dma_start(wk[k][:], wk_d.ap()[k])
            for k in range(EC):
                nc.gpsimd.dma_start(wv[k][:], wv_d.ap()[k])
            nc.gpsimd.dma_start(bvb[:], bvb_d.ap())
            nc.scalar.dma_start(mask[:], mask_d.ap())
            for k in range(DC // 128):
                nc.scalar.dma_start(wo[k][:], wo_d.ap()[k])

            # ---- PE warm-up while the first DMA chunks land ----
            wu = cp.tile([64, 512], BF, tag="wu", name="wu")
            nc.vector.memset(wu[:], 0.125)
            for _ in range(10):
                pw = ps_w.tile([128, 512], F32, tag="psw", name="psw")
                nc.tensor.matmul(pw[0:64, :], wu[:, 0:64], wu[:],
                                 start=True, stop=True)

            ones = cp.tile([65, 64], BF, tag="ones", name="ones")
            nc.any.memset(ones[:], 1.0)

            QT = [cp.tile([128, S], BF, tag=f"QT{t}", name=f"QT{t}") for t in range(4)]
            KT = [cp.tile([128, S], BF, tag=f"KT{t}", name=f"KT{t}") for t in range(4)]
            V = [cp.tile([128, HPC, 66], BF, tag=f"V{s}", name=f"V{s}") for s in range(TT)]
            AOT = [cp.tile([128, S], BF, tag=f"AOT{t}", name=f"AOT{t}") for t in range(4)]

            filler = []

            def proj_group(w_sb, b_sb, dst, t, qc):
                def emit():
                    ps = ps_w.tile([128, 512], F32, tag="psw", name="psw")
                    for k in range(EC):
                        nc.tensor.matmul(
                            ps[:],
                            w_sb[k][:, t * 128:(t + 1) * 128],
                            xT[k][:, qc * 512:(qc + 1) * 512],
                            start=(k == 0), stop=(k == EC - 1))
                    nc.vector.tensor_scalar(
                        dst[t][:, qc * 512:(qc + 1) * 512], ps[:],
                        b_sb[:, t:t + 1], None, ALU.add)
                return emit

            def v_group(s):
                def emit():
                    ps = ps_w.tile([128, 512], F32, tag="psw", name="psw")
                    for k in range(EC):
                        nc.tensor.matmul(
                            ps[:],
                            xT[k][:, s * 128:(s + 1) * 128],
                            wv[k][:],
                            start=(k == 0), stop=(k == EC - 1))
                    nc.vector.tensor_tensor(
                        V[s][:, :, 0:64],
                        ps[:].rearrange("p (h d) -> p h d", d=64),
                        bvb[:].rearrange("p (h d) -> p h d", d=64),
                        ALU.add)
                    nc.any.memset(V[s][:, :, 64:65], 1.0)
                return emit

            def d_group(s):
                def emit():
                    osb = wp.tile([128, E], BF, tag="osb", name="osb")
                    for n in range(2):
                        ps = ps_w.tile([128, 512], F32, tag="psw", name="psw")
                        for k in range(DC // 128):
                            nc.tensor.matmul(
                                ps[:],
                                AOT[k][:, s * 128:(s + 1) * 128],
                                wo[k][:, n * 512:(n + 1) * 512],
                                start=(k == 0), stop=(k == DC // 128 - 1))
                        nc.vector.tensor_copy(out=osb[:, n * 512:(n + 1) * 512],
                                              in_=ps[:])
                    nc.sync.dma_start(out_d.ap()[s], osb[:])
                return emit

            # ---- lead-in: work that must precede phase (0,0); covers the
            # input-DMA window and warms HAM ----
            for qc in range(QCH):
                proj_group(wq, bq, QT, 0, qc)()
                proj_group(wk, bk, KT, 0, qc)()
            for s in range(4):
                v_group(s)()

            # ---- filler queue (deadline-ordered emission) ----
            for hp in range(1, 4):
                for qc in range(QCH):
                    filler.append(("qkt", hp, proj_group(wq, bq, QT, hp, qc)))
                    filler.append(("qkt", hp, proj_group(wk, bk, KT, hp, qc)))
            # order the qkt fillers by deadline phase of hp: 1 -> idx2, 2 ->
            # idx6, 3 -> idx12; v fillers get spliced in at band deadlines
            vq = {qc: [("v", 4 * qc + i, v_group(4 * qc + i)) for i in range(4)]
                  for qc in range(1, 4)}
            # interleave: qkt1(8) | v band1(4) | qkt2(8) | v band2(4) | qkt3(8) | v band3(4)
            fl = []
            fl += [f for f in filler if f[1] == 1]
            fl += vq[1]
            fl += [f for f in filler if f[1] == 2]
            fl += vq[2]
            fl += [f for f in filler if f[1] == 3]
            fl += vq[3]
            filler = fl

            FILL_NS = 1750.0
            debt = [0.0]
            pending_norm = [None]

            def flush_norm():
                if pending_norm[0] is not None:
                    pending_norm[0]()
                    pending_norm[0] = None

            def act_ns(nfree):
                return (nfree + 352) / 1.2

            def emit_filler_until(pred_drop):
                keep = []
                for item in filler:
                    if pred_drop(item):
                        item[2]()
                        debt[0] -= FILL_NS
                    else:
                        keep.append(item)
                filler[:] = keep

            def d_filler(s):
                def emit():
                    flush_norm()
                    d_group(s)()
                return emit

            qc_done = {qc: 0 for qc in range(QCH)}

            for hp, qc in PHASES:
                nkb = 4 * qc + 4
                # data deps: this phase needs QT/KT[hp] complete
                emit_filler_until(lambda it: it[0] == "qkt" and it[1] <= hp)
                hA, hB = 2 * hp, 2 * hp + 1
                pav = {}
                pav[hA] = ps_av.tile([128, 512], F32, tag="pav", name="pav")
                pav[hB] = ps_av.tile([128, 512], F32, tag="pav", name="pav")

                def emit_av(ex, kbs_offs):
                    for h in (hA, hB):
                        for i, kb, off in kbs_offs:
                            nc.tensor.matmul(
                                pav[h][0:65, off:512],
                                V[kb][:, h, 0:65],
                                ex[h][:, i, off:512],
                                start=(kb == 0), stop=(kb == nkb - 1))

                pend = None
                for s0 in range(0, nkb, 2):
                    kbs = list(range(s0, min(s0 + 2, nkb)))
                    emit_filler_until(
                        lambda it: it[0] == "v" and it[1] <= kbs[-1])
                    pss = {h: ps_s.tile([128, 2, 512], F32, tag="pss",
                                        name="pss")
                           for h in (hA, hB)}
                    ex = {h: expp.tile([128, 2, 512], BF,
                                       tag=f"ex{h % 2}", name="ex")
                          for h in (hA, hB)}
                    # scores matmuls write the whole round width [off0:512] so
                    # ONE exp per (round, head) reads only data written this
                    # round (halves diagonal-round ACTIVATE count; act fixed
                    # cost is 352 cycles).  The widened slice of the second kb
                    # holds above-diagonal values nothing reads: AV reads
                    # [off:512] per kb, mask covers the boundary.
                    offs = {}
                    off0 = 128 * (kbs[0] - 4 * qc) if kbs[0] > 4 * qc else 0
                    for i, kb in enumerate(kbs):
                        dj = kb - 4 * qc
                        offs[kb] = 128 * dj if dj > 0 else 0
                        for h, r in ((hA, 0), (hB, 64)):
                            nc.tensor.matmul(
                                pss[h][:, i, off0:512],
                                KT[hp][r:r + 64, kb * 128:(kb + 1) * 128],
                                QT[hp][r:r + 64,
                                       qc * 512 + off0:(qc + 1) * 512],
                                start=True, stop=True)
                    for h in (hA, hB):
                        nc.scalar.activation(
                            ex[h][:, 0:len(kbs), off0:512],
                            pss[h][:, 0:len(kbs), off0:512],
                            AF.Exp, scale=0.125)
                        eng = nc.vector if h == hA else nc.gpsimd
                        for i, kb in enumerate(kbs):
                            if kb - 4 * qc >= 0:
                                off = offs[kb]
                                eng.tensor_tensor(
                                    ex[h][:, i, off:off + 128],
                                    ex[h][:, i, off:off + 128],
                                    mask[:], ALU.mult)
                    sc_free = len(kbs) * (512 - off0)
                    debt[0] += 2 * act_ns(sc_free) \
                        - (0.55 * sc_free + 2 * sc_free) / 2.4
                    while debt[0] >= FILL_NS and filler:
                        filler.pop(0)[2]()
                        debt[0] -= FILL_NS
                    flush_norm()
                    if pend is not None:
                        emit_av(*pend)
                    pend = (ex, [(i, kb, offs[kb])
                                 for i, kb in enumerate(kbs)])
                emit_av(*pend)

                # defer this phase's normalization into the next phase's
                # first round: the den-cast -> broadcast chain otherwise
                # stalls the PE ~1.1us at every phase boundary
                def make_norm(qc, hp, hA, hB, pav):
                    def emit():
                        for h, r in ((hA, 0), (hB, 64)):
                            den = wp.tile([1, 512], BF, tag="den",
                                          name="den")
                            nc.vector.tensor_copy(out=den[:],
                                                  in_=pav[h][64:65, :])
                            psb = ps_w.tile([128, 512], F32, tag="psw",
                                            name="psw")
                            nc.tensor.matmul(psb[0:64, :], ones[0:1, :],
                                             den[:], start=True, stop=True)
                            rcpb = wp.tile([64, 512], F32, tag="rcpb",
                                           name="rcpb")
                            nc.vector.reciprocal_approx_fast(
                                out=rcpb[:], in_=psb[0:64, :])
                            dst = AOT[hp][r:r + 64,
                                          qc * 512:(qc + 1) * 512]
                            nc.vector.tensor_tensor(dst, pav[h][0:64, :],
                                                    rcpb[:], ALU.mult)
                    return emit
                pending_norm[0] = make_norm(qc, hp, hA, hB, pav)

                qc_done[qc] += 1
                if qc_done[qc] == 4:
                    # out-projections for this qc band become available
                    for s in range(qc * 4, qc * 4 + 4):
                        filler.append(("d", s, d_filler(s)))
            flush_norm()
            emit_filler_until(lambda it: True)

    nc.compile()
    return nc


def _get_nc():
    if "nc" not in _CACHE:
        _CACHE["nc"] = _build()
    return _CACHE["nc"]


def _shard_inputs(x, Wq, bq, Wk, bk, Wv, bv, Wo):
    """Build the 8 per-core input maps (host-side shard/cast/transpose)."""
    x = np.asarray(x, np.float32)
    mask = np.triu(np.ones((128, 128), np.float32)).astype(BF16)  # [k, q] q>=k
    in_maps = []
    for c in range(8):
        b, hg = divmod(c, 2)
        dc = slice(hg * DC, (hg + 1) * DC)
        xT = np.ascontiguousarray(x[b].T).astype(BF16).reshape(EC, 128, S)
        wq_c = np.ascontiguousarray(Wq[:, dc]).astype(BF16).reshape(EC, 128, DC)
        wk_c = np.ascontiguousarray(Wk[:, dc]).astype(BF16).reshape(EC, 128, DC)
        wv_c = np.ascontiguousarray(Wv[:, dc]).astype(BF16).reshape(EC, 128, DC)
        wo_c = np.ascontiguousarray(Wo[dc, :]).astype(BF16).reshape(DC // 128, 128, E)
        bq_c = np.ascontiguousarray(np.asarray(bq[dc], np.float32).reshape(4, 128).T)
        bk_c = np.ascontiguousarray(np.asarray(bk[dc], np.float32).reshape(4, 128).T)
        bvb_c = np.ascontiguousarray(
            np.tile(np.asarray(bv[dc], np.float32).reshape(1, DC), (128, 1)))
        in_maps.append({
            "xT": xT, "wq": wq_c, "wk": wk_c, "wv": wv_c, "wo": wo_c,
            "bq": bq_c, "bk": bk_c, "bvb": bvb_c, "mask": mask,
        })
    return in_maps


def kernel(x, Wq, bq, Wk, bk, Wv, bv, Wo, bo):
    from concourse.bass_utils import run_bass_kernel_spmd

    nc = _get_nc()
    in_maps = _shard_inputs(x, Wq, bq, Wk, bk, Wv, bv, Wo)
    res = run_bass_kernel_spmd(nc, in_maps, core_ids=list(range(8)))
    bo = np.asarray(bo, np.float32)
    out = np.empty((B, S, E), np.float32)
    for b in range(B):
        p0 = res.results[2 * b]["out"].reshape(S, E).astype(np.float32)
        p1 = res.results[2 * b + 1]["out"].reshape(S, E).astype(np.float32)
        out[b] = p0 + p1 + bo
    return out


# revision 5
# speedup vs baseline: 1.1113x; 1.1021x over previous
"""Multi-head self-attention (B=4, S=2048, E=1024, H=16, causal) on 8 NeuronCores.

v3: snake phase order over (head-pair, query-chunk) + fine-grained filler
deadlines.  Q/K projection filler groups are split per (hp, qc-chunk) --
phase (hp,qc) only needs QT[hp] chunk qc and KT[hp] chunks 0..qc, so with
the snake order each phase force-emits at most one q+k chunk pair
(3.4us) instead of a whole head-pair (13.7us).  wq/wk are stored t-major
([t][e,128] per e-chunk) so each chunk group gates on its own DMA; xT is
DMA'd per (qc-band, e-chunk) so the lead-in projections pipeline with the
input stream.  Carried-over: attn@V one round behind scores/exp; exp only
on ScalarE; V bias folded into V; causal mask as post-exp multiply (hA on
VectorE, hB on GpSimd); denominator broadcast by rank-1 PE matmul +
custom-DVE reciprocal at base partition 0; debt-metered filler pacing.
"""

import numpy as np
import ml_dtypes

B, S, E, H, D = 4, 2048, 1024, 16, 64
HPC = 8          # heads per core
DC = HPC * D     # 512 sharded feature cols per core
EC = E // 128    # 8 e-chunks
TT = S // 128    # 16 token tiles
QCH = S // 512   # 4 query chunks
NB = S // 128    # 16 key blocks

BF16 = ml_dtypes.bfloat16

_CACHE = {}

# snake order over (hp, qc)
PHASES = [(0, 0), (0, 1), (1, 0), (1, 1), (0, 2), (1, 2), (2, 0), (2, 1),
          (2, 2), (0, 3), (1, 3), (2, 3), (3, 0), (3, 1), (3, 2), (3, 3)]
PIDX = {p: i for i, p in enumerate(PHASES)}


def _build():
    import concourse.tile as tile
    from concourse import bacc, mybir

    F32 = mybir.dt.float32
    BF = mybir.dt.bfloat16
    AF = mybir.ActivationFunctionType
    ALU = mybir.AluOpType

    nc = bacc.Bacc("TRN2", target_bir_lowering=False, debug=False, num_devices=8)

    xT_d = nc.dram_tensor("xT", [QCH, EC, 128, 512], BF, kind="ExternalInput")
    wq_d = nc.dram_tensor("wq", [4, 128, EC * 128], BF, kind="ExternalInput")
    wk_d = nc.dram_tensor("wk", [4, 128, EC * 128], BF, kind="ExternalInput")
    wv_d = nc.dram_tensor("wv", [EC, 128, DC], BF, kind="ExternalInput")
    wo_d = nc.dram_tensor("wo", [DC // 128, 128, E], BF, kind="ExternalInput")
    bq_d = nc.dram_tensor("bq", [128, 4], F32, kind="ExternalInput")
    bk_d = nc.dram_tensor("bk", [128, 4], F32, kind="ExternalInput")
    bvb_d = nc.dram_tensor("bvb", [128, DC], F32, kind="ExternalInput")
    mask_d = nc.dram_tensor("mask", [128, 128], BF, kind="ExternalInput")
    out_d = nc.dram_tensor("out", [TT, 128, E], BF, kind="ExternalOutput")

    with tile.TileContext(nc) as tc:
        with tc.tile_pool(name="const", bufs=1) as cp, \
             tc.tile_pool(name="expp", bufs=4) as expp, \
             tc.tile_pool(name="work", bufs=2) as wp, \
             tc.tile_pool(name="ps_s", bufs=2, space="PSUM") as ps_s, \
             tc.tile_pool(name="ps_av", bufs=2, space="PSUM") as ps_av, \
             tc.tile_pool(name="ps_w", bufs=2, space="PSUM") as ps_w:

            # ---- persistent SBUF tensors ----
            xT = [cp.tile([128, S], BF, tag=f"xT{k}", name=f"xT{k}") for k in range(EC)]
            # t-major weights: wqT[t] cols = (e-chunk k)*128 + dc_sub
            wqT = [cp.tile([128, EC * 128], BF, tag=f"wq{t}", name=f"wq{t}") for t in range(4)]
            wkT = [cp.tile([128, EC * 128], BF, tag=f"wk{t}", name=f"wk{t}") for t in range(4)]
            wv = [cp.tile([128, DC], BF, tag=f"wv{k}", name=f"wv{k}") for k in range(EC)]
            wo = [cp.tile([128, E], BF, tag=f"wo{k}", name=f"wo{k}") for k in range(DC // 128)]
            bq = cp.tile([128, 4], F32, tag="bq", name="bq")
            bk = cp.tile([128, 4], F32, tag="bk", name="bk")
            bvb = cp.tile([128, DC], F32, tag="bvb", name="bvb")
            mask = cp.tile([128, 128], BF, tag="mask", name="mask")

            # ---- input DMAs on the 3 DMA-capable queues, FIFO-ordered by
            # first consumption ----
            nc.sync.dma_start(bq[:], bq_d.ap())
            nc.sync.dma_start(bk[:], bk_d.ap())
            for qc in range(2):
                for k in range(EC):
                    nc.sync.dma_start(xT[k][:, qc * 512:(qc + 1) * 512],
                                      xT_d.ap()[qc][k])
            for t in range(4):
                nc.gpsimd.dma_start(wqT[t][:], wq_d.ap()[t])
                nc.gpsimd.dma_start(wkT[t][:], wk_d.ap()[t])
            for qc in range(2, 4):
                for k in range(EC):
                    nc.scalar.dma_start(xT[k][:, qc * 512:(qc + 1) * 512],
                                        xT_d.ap()[qc][k])
            nc.gpsimd.dma_start(bvb[:], bvb_d.ap())
            for k in range(EC):
                nc.gpsimd.dma_start(wv[k][:], wv_d.ap()[k])
            nc.scalar.dma_start(mask[:], mask_d.ap())
            for k in range(DC // 128):
                nc.scalar.dma_start(wo[k][:], wo_d.ap()[k])

            # ---- PE warm-up fillers (dep-free; popped by debt early on) ----
            wu = cp.tile([64, 512], BF, tag="wu", name="wu")
            nc.vector.memset(wu[:], 0.125)

            def warm_mm():
                pw = ps_w.tile([128, 512], F32, tag="psw", name="psw")
                nc.tensor.matmul(pw[0:64, :], wu[:, 0:64], wu[:],
                                 start=True, stop=True)
            for _ in range(8):
                warm_mm()
            warm_left = [16]

            ones = cp.tile([65, 64], BF, tag="ones", name="ones")
            nc.any.memset(ones[:], 1.0)

            QT = [cp.tile([128, S], BF, tag=f"QT{t}", name=f"QT{t}") for t in range(4)]
            KT = [cp.tile([128, S], BF, tag=f"KT{t}", name=f"KT{t}") for t in range(4)]
            V = [cp.tile([128, HPC, 66], BF, tag=f"V{s}", name=f"V{s}") for s in range(TT)]
            AOT = [cp.tile([128, S], BF, tag=f"AOT{t}", name=f"AOT{t}") for t in range(4)]

            def proj_group(wT, b_sb, dst, t, qc):
                def emit():
                    ps = ps_w.tile([128, 512], F32, tag="psw", name="psw")
                    for k in range(EC):
                        nc.tensor.matmul(
                            ps[:],
                            wT[t][:, k * 128:(k + 1) * 128],
                            xT[k][:, qc * 512:(qc + 1) * 512],
                            start=(k == 0), stop=(k == EC - 1))
                    nc.vector.tensor_scalar(
                        dst[t][:, qc * 512:(qc + 1) * 512], ps[:],
                        b_sb[:, t:t + 1], None, ALU.add)
                return emit

            def v_group(s):
                def emit():
                    ps = ps_w.tile([128, 512], F32, tag="psw", name="psw")
                    for k in range(EC):
                        nc.tensor.matmul(
                            ps[:],
                            xT[k][:, s * 128:(s + 1) * 128],
                            wv[k][:],
                            start=(k == 0), stop=(k == EC - 1))
                    nc.vector.tensor_tensor(
                        V[s][:, :, 0:64],
                        ps[:].rearrange("p (h d) -> p h d", d=64),
                        bvb[:].rearrange("p (h d) -> p h d", d=64),
                        ALU.add)
                    nc.any.memset(V[s][:, :, 64:65], 1.0)
                return emit

            def d_group(s):
                def emit():
                    osb = wp.tile([128, E], BF, tag="osb", name="osb")
                    for n in range(2):
                        ps = ps_w.tile([128, 512], F32, tag="psw", name="psw")
                        for k in range(DC // 128):
                            nc.tensor.matmul(
                                ps[:],
                                AOT[k][:, s * 128:(s + 1) * 128],
                                wo[k][:, n * 512:(n + 1) * 512],
                                start=(k == 0), stop=(k == DC // 128 - 1))
                        nc.vector.tensor_copy(out=osb[:, n * 512:(n + 1) * 512],
                                              in_=ps[:])
                    nc.sync.dma_start(out_d.ap()[s], osb[:])
                return emit

            # ---- lead-in: only what phase (0,0) needs ----
            proj_group(wqT, bq, QT, 0, 0)()
            proj_group(wkT, bk, KT, 0, 0)()

            # ---- filler queue, sorted by (deadline_phase, deadline_round).
            # qkt (hp,qc) chunk pair due at phase (hp,qc) start; v[s] due at
            # the round touching kb=s in the first phase of its band; d(s)
            # appended when its band completes (due at the end). ----
            filler = []
            for hp in range(4):
                for qc in range(QCH):
                    if (hp, qc) == (0, 0):
                        continue
                    pi = PIDX[(hp, qc)]
                    filler.append(((pi, -2), "qkt",
                                   proj_group(wqT, bq, QT, hp, qc)))
                    filler.append(((pi, -2), "qkt",
                                   proj_group(wkT, bk, KT, hp, qc)))
            for s in range(TT):
                band = s // 4
                pi = PIDX[(0, band)]
                filler.append(((pi, s), "v", v_group(s)))
            filler.sort(key=lambda it: it[0])

            FILL_NS = 1750.0
            debt = [0.0]
            pending_norm = [None]

            def flush_norm():
                if pending_norm[0] is not None:
                    pending_norm[0]()
                    pending_norm[0] = None

            def act_ns(nfree):
                return (nfree + 352) / 1.2

            def force_fillers(key):
                while filler and filler[0][0] <= key:
                    filler.pop(0)[2]()
                    debt[0] -= FILL_NS

            def pop_debt():
                while debt[0] >= FILL_NS:
                    if warm_left[0] > 0 and (not filler or filler[0][1] == "v"):
                        # early on, prefer dep-free warm matmuls over pulling
                        # v-groups whose DMA may not have landed
                        warm_mm()
                        warm_left[0] -= 1
                        debt[0] -= 220.0
                        continue
                    if not filler:
                        return
                    filler.pop(0)[2]()
                    debt[0] -= FILL_NS

            def d_filler(s):
                def emit():
                    flush_norm()
                    d_group(s)()
                return emit

            qc_done = {qc: 0 for qc in range(QCH)}

            for pi, (hp, qc) in enumerate(PHASES):
                nkb = 4 * qc + 4
                force_fillers((pi, -2))
                hA, hB = 2 * hp, 2 * hp + 1
                pav = {}
                pav[hA] = ps_av.tile([128, 512], F32, tag="pav", name="pav")
                pav[hB] = ps_av.tile([128, 512], F32, tag="pav", name="pav")

                def emit_av(ex, kbs_offs):
                    for h in (hA, hB):
                        for i, kb, off in kbs_offs:
                            nc.tensor.matmul(
                                pav[h][0:65, off:512],
                                V[kb][:, h, 0:65],
                                ex[h][:, i, off:512],
                                start=(kb == 0), stop=(kb == nkb - 1))

                pend = None
                for s0 in range(0, nkb, 2):
                    kbs = list(range(s0, min(s0 + 2, nkb)))
                    force_fillers((pi, kbs[-1]))
                    pop_debt()
                    pss = {h: ps_s.tile([128, 2, 512], F32, tag="pss",
                                        name="pss")
                           for h in (hA, hB)}
                    ex = {h: expp.tile([128, 2, 512], BF,
                                       tag=f"ex{h % 2}", name="ex")
                          for h in (hA, hB)}
                    # scores matmuls write the whole round width [off0:512] so
                    # ONE exp per (round, head) reads only data written this
                    # round.  The widened slice of the second kb holds
                    # above-diagonal values nothing reads.
                    offs = {}
                    off0 = 128 * (kbs[0] - 4 * qc) if kbs[0] > 4 * qc else 0
                    for i, kb in enumerate(kbs):
                        dj = kb - 4 * qc
                        offs[kb] = 128 * dj if dj > 0 else 0
                        for h, r in ((hA, 0), (hB, 64)):
                            nc.tensor.matmul(
                                pss[h][:, i, off0:512],
                                KT[hp][r:r + 64, kb * 128:(kb + 1) * 128],
                                QT[hp][r:r + 64,
                                       qc * 512 + off0:(qc + 1) * 512],
                                start=True, stop=True)
                    for h in (hA, hB):
                        nc.scalar.activation(
                            ex[h][:, 0:len(kbs), off0:512],
                            pss[h][:, 0:len(kbs), off0:512],
                            AF.Exp, scale=0.125)
                        eng = nc.vector if h == hA else nc.gpsimd
                        for i, kb in enumerate(kbs):
                            if kb - 4 * qc >= 0:
                                off = offs[kb]
                                eng.tensor_tensor(
                                    ex[h][:, i, off:off + 128],
                                    ex[h][:, i, off:off + 128],
                                    mask[:], ALU.mult)
                    sc_free = len(kbs) * (512 - off0)
                    debt[0] += 2 * act_ns(sc_free) \
                        - (0.55 * sc_free + 2 * sc_free) / 2.4
                    flush_norm()
                    if pend is not None:
                        emit_av(*pend)
                    pend = (ex, [(i, kb, offs[kb])
                                 for i, kb in enumerate(kbs)])
                emit_av(*pend)

                # defer this phase's normalization into the next phase's
                # first round (the den-cast -> broadcast chain otherwise
                # stalls the PE at every phase boundary)
                def make_norm(qc, hp, hA, hB, pav):
                    def emit():
                        for h, r in ((hA, 0), (hB, 64)):
                            den = wp.tile([1, 512], BF, tag="den",
                                          name="den")
                            nc.vector.tensor_copy(out=den[:],
                                                  in_=pav[h][64:65, :])
                            psb = ps_w.tile([128, 512], F32, tag="psw",
                                            name="psw")
                            nc.tensor.matmul(psb[0:64, :], ones[0:1, :],
                                             den[:], start=True, stop=True)
                            rcpb = wp.tile([64, 512], F32, tag="rcpb",
                                           name="rcpb")
                            nc.vector.reciprocal_approx_fast(
                                out=rcpb[:], in_=psb[0:64, :])
                            dst = AOT[hp][r:r + 64,
                                          qc * 512:(qc + 1) * 512]
                            nc.vector.tensor_tensor(dst, pav[h][0:64, :],
                                                    rcpb[:], ALU.mult)
                    return emit
                pending_norm[0] = make_norm(qc, hp, hA, hB, pav)

                qc_done[qc] += 1
                if qc_done[qc] == 4:
                    for s in range(qc * 4, qc * 4 + 4):
                        filler.append(((len(PHASES), s), "d", d_filler(s)))
            flush_norm()
            force_fillers((len(PHASES) + 1, 99))

    nc.compile()
    return nc


def _get_nc():
    if "nc" not in _CACHE:
        _CACHE["nc"] = _build()
    return _CACHE["nc"]


def _shard_inputs(x, Wq, bq, Wk, bk, Wv, bv, Wo):
    """Build the 8 per-core input maps (host-side shard/cast/transpose)."""
    x = np.asarray(x, np.float32)
    mask = np.triu(np.ones((128, 128), np.float32)).astype(BF16)  # [k, q] q>=k
    in_maps = []
    for c in range(8):
        b, hg = divmod(c, 2)
        dc = slice(hg * DC, (hg + 1) * DC)
        # xT[qc, k, e, s'] = x[b].T[k*128+e, qc*512+s']
        xT = np.ascontiguousarray(x[b].T).astype(BF16)
        xT = xT.reshape(EC, 128, QCH, 512).transpose(2, 0, 1, 3)
        xT = np.ascontiguousarray(xT)
        # wq[t, e, k*128+d] = Wq[k*128+e, dc0 + t*128+d]
        wq_c = np.asarray(Wq[:, dc], np.float32).reshape(EC, 128, 4, 128)
        wq_c = np.ascontiguousarray(wq_c.transpose(2, 1, 0, 3).reshape(
            4, 128, EC * 128)).astype(BF16)
        wk_c = np.asarray(Wk[:, dc], np.float32).reshape(EC, 128, 4, 128)
        wk_c = np.ascontiguousarray(wk_c.transpose(2, 1, 0, 3).reshape(
            4, 128, EC * 128)).astype(BF16)
        wv_c = np.ascontiguousarray(Wv[:, dc]).astype(BF16).reshape(EC, 128, DC)
        wo_c = np.ascontiguousarray(Wo[dc, :]).astype(BF16).reshape(DC // 128, 128, E)
        bq_c = np.ascontiguousarray(np.asarray(bq[dc], np.float32).reshape(4, 128).T)
        bk_c = np.ascontiguousarray(np.asarray(bk[dc], np.float32).reshape(4, 128).T)
        bvb_c = np.ascontiguousarray(
            np.tile(np.asarray(bv[dc], np.float32).reshape(1, DC), (128, 1)))
        in_maps.append({
            "xT": xT, "wq": wq_c, "wk": wk_c, "wv": wv_c, "wo": wo_c,
            "bq": bq_c, "bk": bk_c, "bvb": bvb_c, "mask": mask,
        })
    return in_maps


def kernel(x, Wq, bq, Wk, bk, Wv, bv, Wo, bo):
    from concourse.bass_utils import run_bass_kernel_spmd

    nc = _get_nc()
    in_maps = _shard_inputs(x, Wq, bq, Wk, bk, Wv, bv, Wo)
    res = run_bass_kernel_spmd(nc, in_maps, core_ids=list(range(8)))
    bo = np.asarray(bo, np.float32)
    out = np.empty((B, S, E), np.float32)
    for b in range(B):
        p0 = res.results[2 * b]["out"].reshape(S, E).astype(np.float32)
        p1 = res.results[2 * b + 1]["out"].reshape(S, E).astype(np.float32)
        out[b] = p0 + p1 + bo
    return out
